# revision 1
# baseline (speedup 1.0000x reference)
"""GCC-PHAT Trainium2 kernel (v2: fp16 datapath).

Pipeline (per core, batch-sharded B=16 -> 2 per core):
  1. Forward rfft as PE matmul in fp16 (fp32 PSUM accumulate):
     xT[b,m,n,t] (host-pretransposed, fp16) @ F[1024,1024] fp16.
     F cols 0..511 = cos(2pi n f/L) f=1..512; cols 512..1022 = -sin, f=1..511;
     col 1023 = ones (bin-0 sum S). Output X.T in PSUM, f on partitions
     (8 chunks of 128: 4 'a' = Re, 4 'b' = Im(+S in chunk7 row127)).
  2. PHAT normalize per mic: w' = 1/sqrt(16*(a^2+b^2)) via ACT
     Abs_reciprocal_sqrt; Y = X*w' (= unit/4) stored fp16.
     Specials: bin512 row -> sign(a512); S row -> sign(S) = y0.
  3. Pair products (28 mic pairs, 4 planes aa/bb/ab/ba) fp16:
     aa/bb/ab on DVE, ba on GPSIMD; |R| products = 1/16.
  4. Truncated inverse DFT as PE matmul, G STATIONARY: lhsT = G[128f x 64],
     rhs = R planes [128f x (lane,t)], accumulated over 16 (plane,chunk)
     K-chunks into PSUM [64, rows]. G rows carry 16x scale (except the
     bin0/bin512 sign-slot rows, product scale 1) + irfft weights/
     fftshift/slice surgery.
  5. PSUM -> ACT copy -> SBUF -> DMA to out[b, lag, p, t] (lag-major;
     host transposes back to [b, p, t, lag]).
"""

import os
from contextlib import ExitStack

import numpy as np

import concourse.bass as bass
import concourse.bacc as bacc
import concourse.mybir as mybir
import concourse.tile as tile
from concourse.bass import ds, ts
from concourse.bass_utils import run_bass_kernel_spmd

B, M, T, L = 16, 8, 250, 1024
NCORES = 8
NB = B // NCORES          # batches per core
NPAIRS = (M * (M - 1)) // 2   # 28
NL = 64                   # output lags
F32 = mybir.dt.float32
FP16 = mybir.dt.float16


def _build_F() -> np.ndarray:
    n = np.arange(L, dtype=np.float64)[:, None]
    F = np.zeros((L, L), dtype=np.float64)
    f_a = np.arange(1, 513, dtype=np.float64)[None, :]
    f_b = np.arange(1, 512, dtype=np.float64)[None, :]
    F[:, 0:512] = np.cos(2 * np.pi * n * f_a / L)
    F[:, 512:1023] = -np.sin(2 * np.pi * n * f_b / L)
    F[:, 1023] = 1.0
    return F.astype(np.float16)


def _build_G() -> np.ndarray:
    G = np.zeros((13, 128, NL), dtype=np.float64)
    nj = (np.arange(NL) - 32).astype(np.float64)
    for c in range(4):
        for r in range(128):
            f = 128 * c + r + 1
            w = 1.0 if f == 512 else 2.0
            cosv = 16.0 * w * np.cos(2 * np.pi * f * nj / L) / L
            sinv = 16.0 * w * np.sin(2 * np.pi * f * nj / L) / L
            if c < 3:
                G[0 + c, r] = cosv - sinv     # k1 = (a1+b1)*a2
                G[3 + c, r] = sinv            # k2' = a1*(a2+b2)
                G[6 + c, r] = -cosv           # k3 = b1*(a2-b2)
            else:
                G[9, r] = cosv                # aa
                G[10, r] = cosv               # bb
                G[11, r] = sinv               # ab
                G[12, r] = -sinv              # ba
    # sign-slot rows (chunk3 row127): products carry scale 1 (not 1/16)
    G[9, 127, :] = np.cos(np.pi * nj) / L     # sign(a512) products
    G[10, 127, :] = 1.0 / L                   # y0 products (bin 0)
    G[11, 127, :] = 0.0
    G[12, 127, :] = 0.0
    return G.astype(np.float16)


def _pair_subgroups():
    """(m1, mg2, lo, hi, p_base): pairs (m1, m2), m2 = 2*mg2+lo .. 2*mg2+hi-1;
    p_base = global pair index of the subgroup's first pair."""
    def pidx(m1, m2):
        return m1 * (2 * M - m1 - 1) // 2 + (m2 - m1 - 1)
    out = []
    for m1 in range(M - 1):
        for mg2 in range(4):
            lo = max(0, m1 + 1 - 2 * mg2)
            if lo >= 2:
                continue
            m2_0 = 2 * mg2 + lo
            if m2_0 <= m1:
                continue
            out.append((m1, mg2, lo, 2, pidx(m1, m2_0)))
    return out


def build_bass() -> bass.Bass:
    nc = bacc.Bacc("TRN2", target_bir_lowering=False, debug=False)
    xT = nc.dram_tensor("xT", [NB, M, L, T], FP16, kind="ExternalInput")
    out = nc.dram_tensor("out", [NB, NL, NPAIRS, T], F32, kind="ExternalOutput")
    Fh = nc.inline_tensor(_build_F(), name="Fmat")
    Gh = nc.inline_tensor(np.ascontiguousarray(_build_G()), name="Gmat")

    with tile.TileContext(nc) as tc, ExitStack() as ctx:
        consts = ctx.enter_context(tc.tile_pool(name="consts", bufs=1))
        xt_pool = ctx.enter_context(tc.tile_pool(name="xt", bufs=3))
        y_pool = ctx.enter_context(tc.tile_pool(name="y", bufs=1))
        tmp_pool = ctx.enter_context(tc.tile_pool(name="tmp", bufs=2))
        r_pool = ctx.enter_context(tc.tile_pool(name="r", bufs=3))
        fwd_psum = ctx.enter_context(tc.tile_pool(name="fps", bufs=3, space="PSUM"))
        inv_psum = ctx.enter_context(tc.tile_pool(name="ips", bufs=2, space="PSUM"))

        f_sb = consts.tile([128, 8, L], FP16)
        nc.sync.dma_start(f_sb[:], Fh[:].rearrange("(k p) c -> p k c", p=128))
        g_sb = consts.tile([128, 13, NL], FP16)
        nc.sync.dma_start(g_sb[:], Gh[:].rearrange("i p j -> p i j"))

        for b in range(NB):
            # Y tiles: [128, mg(4), m(2), t] fp16 per (chunk, re/im)
            ya = [y_pool.tile([128, 4, 2, T], FP16, tag=f"ya{c}", name=f"ya{c}") for c in range(4)]
            yb = [y_pool.tile([128, 4, 2, T], FP16, tag=f"yb{c}", name=f"yb{c}") for c in range(4)]
            ys = [y_pool.tile([128, 4, 2, T], FP16, tag=f"ys{c}", name=f"ys{c}") for c in range(3)]
            yd = [y_pool.tile([128, 4, 2, T], FP16, tag=f"yd{c}", name=f"yd{c}") for c in range(3)]

            # ---- forward + normalize ----
            for mg in range(4):
                xt_sb = xt_pool.tile([128, 8, 2, T], FP16, tag="xt")
                for mi in range(2):
                    nc.sync.dma_start(
                        xt_sb[:, :, mi],
                        xT[b, 2 * mg + mi].rearrange("(k p) t -> p k t", p=128),
                    )
                for c in range(4):
                    ps_a = fwd_psum.tile([128, 2, T], F32, tag="psa")
                    ps_b = fwd_psum.tile([128, 2, T], F32, tag="psb")
                    for k in range(8):
                        nc.tensor.matmul(
                            ps_a[:],
                            f_sb[:, k, ts(c, 128)],
                            xt_sb[:, k],
                            start=(k == 0), stop=(k == 7),
                        )
                    for k in range(8):
                        nc.tensor.matmul(
                            ps_b[:],
                            f_sb[:, k, ts(4 + c, 128)],
                            xt_sb[:, k],
                            start=(k == 0), stop=(k == 7),
                        )
                    # normalize
                    np_ = 128 if c < 3 else 127  # row127 special on c==3
                    sq_a = tmp_pool.tile([128, 2, T], F32, tag="sqa")
                    sq_b = tmp_pool.tile([128, 2, T], F32, tag="sqb")
                    w = tmp_pool.tile([128, 2, T], F32, tag="w")
                    nc.scalar.square(sq_a[:np_], ps_a[:np_])
                    nc.scalar.square(sq_b[:np_], ps_b[:np_])
                    nc.gpsimd.tensor_add(sq_a[:np_], sq_a[:np_], sq_b[:np_])
                    # w' = 1/sqrt(16*r) = (1/|X|)/4
                    nc.scalar.activation(
                        w[:np_], sq_a[:np_],
                        mybir.ActivationFunctionType.Abs_reciprocal_sqrt,
                        scale=16.0,
                    )
                    if c == 3:
                        # sign() rows 96..127 first; the w-mult below then
                        # overwrites rows 96..126, leaving row127 = sign
                        # (ops must start on a 32-partition boundary).
                        nc.scalar.sign(ya[c][96:128, mg], ps_a[96:128])
                        nc.scalar.sign(yb[c][96:128, mg], ps_b[96:128])
                    nc.vector.tensor_mul(ya[c][:np_, mg], ps_a[:np_], w[:np_])
                    nc.vector.tensor_mul(yb[c][:np_, mg], ps_b[:np_], w[:np_])
                    if c < 3:
                        nc.vector.tensor_add(ys[c][:, mg], ya[c][:, mg], yb[c][:, mg])
                        nc.vector.tensor_sub(yd[c][:, mg], ya[c][:, mg], yb[c][:, mg])

            # ---- pairs + inverse (diagonal pairing, full-T lane groups) ----
            # diagonal d pairs (m, m+d): both operands contiguous mic slices;
            # lane groups of <=4 keep slices even-sized and 500B-aligned so
            # DVE fp16 TTs stay in 2x mode. Output is diag-major in k.
            yaf = [ya[c][:].rearrange("p a b t -> p (a b t)") for c in range(4)]
            ybf = [yb[c][:].rearrange("p a b t -> p (a b t)") for c in range(4)]
            ysf = [ys[c][:].rearrange("p a b t -> p (a b t)") for c in range(3)]
            ydf = [yd[c][:].rearrange("p a b t -> p (a b t)") for c in range(3)]
            for d in range(1, M):
                lanes = M - d
                kb = sum(M - dd for dd in range(1, d))
                for l0 in range(0, lanes, 4):
                    lc = min(4, lanes - l0)
                    rows = lc * T
                    s1 = slice(l0 * T, l0 * T + rows)            # m1 side
                    s2 = slice((l0 + d) * T, (l0 + d) * T + rows)  # m2 side
                    r_sb = r_pool.tile([128, 13, 4 * T], FP16, tag="ru")
                    for c in range(3):
                        nc.vector.tensor_mul(r_sb[:, 0 + c, :rows], ysf[c][:, s1], yaf[c][:, s2])
                        nc.vector.tensor_mul(r_sb[:, 3 + c, :rows], yaf[c][:, s1], ysf[c][:, s2])
                        if c >= 2:
                            nc.vector.tensor_mul(r_sb[:, 6 + c, :rows], ybf[c][:, s1], ydf[c][:, s2])
                        else:
                            nc.gpsimd.tensor_mul(r_sb[:, 6 + c, :rows], ybf[c][:, s1], ydf[c][:, s2])
                    nc.vector.tensor_mul(r_sb[:, 9, :rows], yaf[3][:, s1], yaf[3][:, s2])
                    nc.vector.tensor_mul(r_sb[:, 10, :rows], ybf[3][:, s1], ybf[3][:, s2])
                    nc.vector.tensor_mul(r_sb[:, 11, :rows], yaf[3][:, s1], ybf[3][:, s2])
                    nc.gpsimd.tensor_mul(r_sb[:, 12, :rows], ybf[3][:, s1], yaf[3][:, s2])
                    for n0 in range(0, rows, 500):
                        nn = min(500, rows - n0)
                        ps_o = inv_psum.tile([64, 500], F32, tag="ops")
                        for idx in range(13):
                            nc.tensor.matmul(
                                ps_o[:, :nn],
                                g_sb[:, idx],
                                r_sb[:, idx, ds(n0, nn)],
                                start=(idx == 0), stop=(idx == 12),
                            )
                        o_sb = tmp_pool.tile([64, 2, T], F32, tag="osb")
                        nlanes = nn // T
                        nc.scalar.copy(
                            o_sb[:, :nlanes],
                            ps_o[:, :nn].rearrange("p (l t) -> p l t", t=T),
                        )
                        nc.sync.dma_start(
                            out[b, :, ds(kb + l0 + n0 // T, nlanes)],
                            o_sb[:, :nlanes],
                        )
    nc.compile()
    return nc


_NC_CACHE = None


def kernel(x: np.ndarray) -> np.ndarray:
    global _NC_CACHE
    x = np.asarray(x, dtype=np.float32)
    assert x.shape == (B, M, T, L)
    xT = np.ascontiguousarray(x.transpose(0, 1, 3, 2)).astype(np.float16)
    if _NC_CACHE is None:
        _NC_CACHE = build_bass()
    nc = _NC_CACHE
    in_maps = [{"xT": xT[c * NB:(c + 1) * NB]} for c in range(NCORES)]
    trace = bool(int(os.environ.get("GCC_TRACE", "0")))
    res = run_bass_kernel_spmd(nc, in_maps, core_ids=list(range(NCORES)),
                               trace=trace)
    if trace and res.exec_time_ns is not None:
        print(f"HW exec time: {res.exec_time_ns} ns")
        if res.instructions_and_trace is not None:
            print("trace:", res.instructions_and_trace[1])
    out = np.concatenate([r["out"] for r in res.results], axis=0)  # [B,NL,28diag,T]
    plist = [m * (2 * M - m - 1) // 2 + (m + d - m - 1)
             for d in range(1, M) for m in range(M - d)]
    final = np.empty((B, NPAIRS, T, NL), dtype=np.float32)
    final[:, plist] = out.transpose(0, 2, 3, 1)
    return final

